# revision 5
# baseline (speedup 1.0000x reference)
"""APPNP (MLP + 2-step GCN propagation) on 8 Trainium2 NeuronCores.

Strategy (data-parallel over nodes, per sharding hint):
 - nodes sharded 12500/core (padded to 12544); within each core, nodes are
   relabeled by descending in-degree so the padded-CSR slot count per
   128-node tile stays tight.
 - MLP runs transposed (features on partitions) in bf16, PSUM fp32.
 - propagation: g = dinv * h is AllGathered (bf16) into a replicated DRAM
   table [8*12544, 64]; each core gathers g[src] for its (dst-sorted,
   slot-padded) edges via indirect DMA and segment-sums with
   identity-stationary matmuls accumulating in PSUM.
 - h_{k+1} = 0.9*dinv*(agg + g_own) + 0.1*h0 computed per 128-node tile on
   the Vector engine.
"""

import numpy as np
import ml_dtypes

N, E, F, H, C = 100000, 1600000, 512, 256, 64
KSTEPS, ALPHA = 2, 0.1
M = 8                      # cores
NSH = N // M               # 12500 real nodes per core
NSP = 12544                # padded (98 * 128)
NT = NSP // 128            # 98 tiles per core
TBL = M * NSP              # 100352 table rows
BF16 = ml_dtypes.bfloat16


def _host_prep(x, W1, b1, p, W2, b2, edge_index):
    """Pure index/layout prep on host. Returns per-core input maps plus the
    static slot schedule and inverse permutations."""
    src = edge_index[0].astype(np.int64)
    dst = edge_index[1].astype(np.int64)

    ins_count = np.bincount(dst, minlength=N)          # in-edges per node
    deg = (ins_count + 1).astype(np.float32)           # + self loop

    # per-core degree-descending relabel
    pos = np.empty(N, dtype=np.int64)                  # orig -> padded-local
    perms = []
    for c in range(M):
        lo = c * NSH
        perm = np.argsort(-ins_count[lo:lo + NSH], kind="stable")
        pp = np.empty(NSH, dtype=np.int64)
        pp[perm] = np.arange(NSH)
        pos[lo:lo + NSH] = pp
        perms.append(perm)
    pad_global = (np.arange(N) // NSH) * NSP + pos     # orig -> table row

    src_pad = pad_global[src]
    dst_core = dst // NSH
    dst_pl = pos[dst]                                  # padded-local dst

    # slot schedule: S_t = max in-count within tile t across all cores (>=1)
    cnt_padded = np.zeros(M * NSP, dtype=np.int64)
    np.add.at(cnt_padded, dst_core * NSP + dst_pl, 1)
    S_t = cnt_padded.reshape(M, NT, 128).max(axis=(0, 2))
    S_t = np.maximum(S_t, 1)
    colstart = np.concatenate([[0], np.cumsum(S_t)]).astype(np.int64)
    TC = int(colstart[-1])

    in_maps = []
    for c in range(M):
        sel = dst_core == c
        s_pad_c = src_pad[sel]
        dpl = dst_pl[sel]
        order = np.argsort(dpl, kind="stable")
        dpl_s = dpl[order]
        s_pad_s = s_pad_c[order]
        cnt = np.bincount(dpl_s, minlength=NSP)
        start = np.concatenate([[0], np.cumsum(cnt)])[:-1]
        slot = np.arange(dpl_s.size) - start[dpl_s]
        tiles = dpl_s // 128
        rows = dpl_s % 128
        cols = colstart[tiles] + slot
        zero_row = c * NSP + NSH                       # a zeroed pad row
        idx_arr = np.full((128, TC), zero_row, dtype=np.int32)
        idx_arr[rows, cols] = s_pad_s

        perm = perms[c]
        lo = c * NSH
        xt = np.zeros((F, NSP), dtype=BF16)
        xt[:, :NSH] = np.ascontiguousarray(x[lo + perm, :].T).astype(BF16)
        deg_pad = np.ones(NSP, dtype=np.float32)
        deg_pad[:NSH] = deg[lo + perm]

        in_maps.append({
            "xt": xt,
            "idx": idx_arr,
            "deg": np.ascontiguousarray(deg_pad.reshape(NT, 128).T),
            "w1": W1.astype(BF16),
            "w2": W2.astype(BF16),
            "pvec": p.astype(np.float32).reshape(2, 128).T.copy(),
            "b1": b1.astype(np.float32).reshape(2, 128).T.copy(),
            "b2": b2.astype(np.float32).reshape(64, 1).copy(),
            "ident": np.eye(128, dtype=BF16),
            "identf": np.eye(64, dtype=np.float32),
        })

    sched = {"S_t": [int(s) for s in S_t], "TC": TC}
    return in_maps, sched, perms


def _build(sched):
    import concourse.bacc as bacc
    import concourse.bass as bass
    import concourse.mybir as mybir
    import concourse.tile as tile

    S_t = sched["S_t"]
    TC = sched["TC"]
    fp32 = mybir.dt.float32
    bf16 = mybir.dt.bfloat16
    i32 = mybir.dt.int32

    nc = bacc.Bacc(None, target_bir_lowering=False)

    xt_e = nc.declare_dram_parameter("xt", [F, NSP], bf16, isOutput=False)
    idx_e = nc.declare_dram_parameter("idx", [128, TC], i32, isOutput=False)
    deg_e = nc.declare_dram_parameter("deg", [128, NT], fp32, isOutput=False)
    w1_e = nc.declare_dram_parameter("w1", [F, H], bf16, isOutput=False)
    w2_e = nc.declare_dram_parameter("w2", [H, C], bf16, isOutput=False)
    p_e = nc.declare_dram_parameter("pvec", [128, 2], fp32, isOutput=False)
    b1_e = nc.declare_dram_parameter("b1", [128, 2], fp32, isOutput=False)
    b2_e = nc.declare_dram_parameter("b2", [64, 1], fp32, isOutput=False)
    id_e = nc.declare_dram_parameter("ident", [128, 128], bf16, isOutput=False)
    idf_e = nc.declare_dram_parameter("identf", [64, 64], fp32, isOutput=False)
    out_e = nc.declare_dram_parameter("out", [NSP, C], fp32, isOutput=True)

    # MLP row chunks
    rcs = []
    off = 0
    while off < NSP:
        w = min(512, NSP - off)
        rcs.append((off, w))
        off += w

    with tile.TileContext(nc) as tc:
        with (
            tc.tile_pool(name="const", bufs=1) as constp,
            tc.tile_pool(name="big", bufs=1) as bigp,
            tc.tile_pool(name="xts", bufs=3) as xtp,
            tc.tile_pool(name="acts", bufs=3) as actp,
            tc.tile_pool(name="gat", bufs=4) as gatp,
            tc.tile_pool(name="epi", bufs=8) as epip,
            tc.tile_pool(name="ps1", bufs=2, space="PSUM") as ps1,
            tc.tile_pool(name="ps2", bufs=2, space="PSUM") as ps2,
            tc.tile_pool(name="pst", bufs=2, space="PSUM") as pst,
            tc.tile_pool(name="psa", bufs=2, space="PSUM") as psa,
            tc.tile_pool(name="dram", bufs=1, space="DRAM") as dramp,
        ):
            # ---- constants ----
            w1_sb = constp.tile([128, 4, H], bf16)
            nc.sync.dma_start(out=w1_sb[:], in_=w1_e.ap().rearrange("(c p) h -> p c h", p=128))
            w2_sb = constp.tile([128, 2, C], bf16)
            nc.sync.dma_start(out=w2_sb[:], in_=w2_e.ap().rearrange("(c p) h -> p c h", p=128))
            id_sb = constp.tile([128, 128], bf16)
            nc.sync.dma_start(out=id_sb[:], in_=id_e[:, :])
            idf_sb = constp.tile([64, 64], fp32)
            nc.sync.dma_start(out=idf_sb[:], in_=idf_e[:, :])
            idx_sb = constp.tile([128, TC], i32)
            nc.sync.dma_start(out=idx_sb[:], in_=idx_e[:, :])
            b2_sb = constp.tile([64, 1], fp32)
            nc.sync.dma_start(out=b2_sb[:], in_=b2_e[:, :])

            p_sb = constp.tile([128, 2], fp32)
            nc.sync.dma_start(out=p_sb[:], in_=p_e[:, :])
            b1_sb = constp.tile([128, 2], fp32)
            nc.sync.dma_start(out=b1_sb[:], in_=b1_e[:, :])
            pc_sb = constp.tile([128, 2], fp32)
            nc.vector.tensor_scalar(
                out=pc_sb[:], in0=p_sb[:], scalar1=0.0, scalar2=1.0,
                op0=mybir.AluOpType.max, op1=mybir.AluOpType.min)
            pb1_sb = constp.tile([128, 2], fp32)
            nc.vector.tensor_mul(out=pb1_sb[:], in0=pc_sb[:], in1=b1_sb[:])

            deg_sb = constp.tile([128, NT], fp32)
            nc.sync.dma_start(out=deg_sb[:], in_=deg_e[:, :])
            sq_sb = constp.tile([128, NT], fp32)
            nc.scalar.sqrt(out=sq_sb[:], in_=deg_sb[:])
            dinv_sb = constp.tile([128, NT], fp32)
            nc.vector.reciprocal(out=dinv_sb[:], in_=sq_sb[:])
            dinv09_sb = constp.tile([128, NT], fp32)
            nc.vector.tensor_scalar_mul(dinv09_sb[:], dinv_sb[:], 1.0 - ALPHA)

            # ---- persistent big buffers ----
            h0a_sb = bigp.tile([128, NT, C], fp32)     # 0.1 * h0
            g_sb = bigp.tile([128, NT, C], bf16)       # current g (own shard)

            # ---- DRAM bounce + tables ----
            bounce = [dramp.tile([NSP, C], bf16, tag=f"bounce{k}", name=f"bounce{k}") for k in range(2)]
            table = [
                dramp.tile([TBL, C], bf16, tag=f"table{k}", name=f"table{k}", addr_space="Shared")
                for k in range(2)
            ]

            # zero the 44 pad rows (12500..12543) of both bounces once, up
            # front; per-tile bounce writes below stop at row NSH so these
            # stay zero.
            zero_sb = constp.tile([128, 64], bf16, name="zero_sb")
            nc.vector.memset(zero_sb[:], 0.0)
            for k in range(2):
                nc.sync.dma_start(out=bounce[k][NSH:NSP, :], in_=zero_sb[:44, :])

            def bounce_tile(k, t):
                # stream this tile's g rows into bounce[k] on the idle sync
                # queue, overlapped under PE/Pool work (row t*128+p <- g[p,t])
                rows = min(128, NSH - t * 128)
                nc.sync.dma_start(
                    out=bounce[k][t * 128:t * 128 + rows, :],
                    in_=g_sb[:rows, t, :])

            # ================= MLP =================
            for (off, w) in rcs:
                xt_sb = xtp.tile([128, 4, 512], bf16, tag="xt")
                nc.sync.dma_start(
                    out=xt_sb[:, :, :w],
                    in_=xt_e[:, off:off + w].rearrange("(c p) n -> p c n", p=128))
                a_sb = actp.tile([128, 2, 512], bf16, tag="a")
                for ht in range(2):
                    pt1 = ps1.tile([128, 512], fp32, tag="pt1")
                    for fc in range(4):
                        nc.tensor.matmul(
                            pt1[:, :w],
                            lhsT=w1_sb[:, fc, ht * 128:(ht + 1) * 128],
                            rhs=xt_sb[:, fc, :w],
                            start=(fc == 0), stop=(fc == 3))
                    nc.scalar.activation(
                        out=a_sb[:, ht, :w], in_=pt1[:, :w],
                        func=mybir.ActivationFunctionType.Relu,
                        bias=pb1_sb[:, ht:ht + 1], scale=pc_sb[:, ht:ht + 1])
                pt2 = ps2.tile([64, 512], fp32, tag="pt2")
                for ht in range(2):
                    nc.tensor.matmul(
                        pt2[:, :w], lhsT=w2_sb[:, ht, :], rhs=a_sb[:, ht, :w],
                        start=(ht == 0), stop=(ht == 1))
                h0t_sb = actp.tile([64, 512], fp32, tag="h0t")
                nc.vector.tensor_scalar_add(h0t_sb[:, :w], pt2[:, :w], b2_sb[:, :1])
                for j in range(w // 128):
                    t = off // 128 + j
                    ptt = pst.tile([128, 64], fp32, tag="ptt")
                    nc.tensor.transpose(
                        out=ptt[:], in_=h0t_sb[:, j * 128:(j + 1) * 128],
                        identity=idf_sb[:])
                    nc.vector.tensor_scalar_mul(h0a_sb[:, t, :], ptt[:], ALPHA)
                    nc.vector.tensor_scalar_mul(g_sb[:, t, :], ptt[:], dinv_sb[:, t:t + 1])
                    bounce_tile(0, t)

            def emit_ag(k):
                nc.gpsimd.collective_compute(
                    "AllGather", mybir.AluOpType.bypass,
                    replica_groups=[list(range(M))],
                    ins=[bounce[k].opt()], outs=[table[k].opt()])

            emit_ag(0)

            # ================= propagation =================
            # column -> (tile, slot) map; columns gathered in groups of 32 via
            # a single indirect DMA with a [128, GW] offset AP (4096
            # descriptors) so the ~1 us SWDGE fixed overhead on the Pool
            # engine is paid once per 32 columns instead of per column.
            colstart = [0]
            for st in S_t:
                colstart.append(colstart[-1] + st)
            col2tile = []
            for t, st in enumerate(S_t):
                col2tile += [t] * st
            GW = 32

            def epilogue(k, t, pag, last):
                # h_new = dinv09*(agg + g_own) + h0a
                t1 = epip.tile([128, 64], fp32, tag="t1")
                nc.vector.tensor_add(t1[:], pag[:], g_sb[:, t, :])
                hn = epip.tile([128, 64], fp32, tag="hn")
                nc.vector.tensor_scalar(
                    out=hn[:], in0=t1[:], scalar1=dinv09_sb[:, t:t + 1],
                    scalar2=None, op0=mybir.AluOpType.mult)
                nc.vector.tensor_add(hn[:], hn[:], h0a_sb[:, t, :])
                if last:
                    nc.sync.dma_start(out=out_e[t * 128:(t + 1) * 128, :], in_=hn[:])
                else:
                    nc.vector.tensor_scalar(
                        out=g_sb[:, t, :], in0=hn[:],
                        scalar1=dinv_sb[:, t:t + 1], scalar2=None,
                        op0=mybir.AluOpType.mult)
                    bounce_tile(k + 1, t)

            for k in range(KSTEPS):
                last = k == KSTEPS - 1
                pag = None
                for g0 in range(0, TC, GW):
                    gw = min(GW, TC - g0)
                    gt8 = gatp.tile([128, GW, 64], bf16, tag="gt")
                    for j in range(gw):
                        nc.gpsimd.indirect_dma_start(
                            out=gt8[:, j, :], out_offset=None,
                            in_=table[k][:, :],
                            in_offset=bass.IndirectOffsetOnAxis(
                                ap=idx_sb[:, g0 + j:g0 + j + 1], axis=0))
                    for j in range(gw):
                        c = g0 + j
                        t = col2tile[c]
                        s = c - colstart[t]
                        if s == 0:
                            pag = psa.tile([128, 64], fp32, tag="pag")
                        nc.tensor.matmul(
                            pag[:], lhsT=id_sb[:], rhs=gt8[:, j, :],
                            start=(s == 0), stop=(s == S_t[t] - 1))
                        if s == S_t[t] - 1:
                            epilogue(k, t, pag, last)
                if not last:
                    emit_ag(1)
    nc.compile()
    return nc


def kernel(x, W1, b1, p, W2, b2, edge_index):
    from concourse.bass_utils import run_bass_kernel_spmd

    x = np.asarray(x)
    in_maps, sched, perms = _host_prep(
        np.asarray(x, dtype=np.float32), np.asarray(W1, dtype=np.float32),
        np.asarray(b1, dtype=np.float32), np.asarray(p, dtype=np.float32),
        np.asarray(W2, dtype=np.float32), np.asarray(b2, dtype=np.float32),
        np.asarray(edge_index))
    nc = _build(sched)
    res = run_bass_kernel_spmd(nc, in_maps, list(range(M)))
    out = np.empty((N, C), dtype=np.float32)
    for c in range(M):
        oc = res.results[c]["out"]
        out[c * NSH + perms[c], :] = oc[:NSH, :]
    return out

